# revision 3
# baseline (speedup 1.0000x reference)
"""CenterLoss on 8 Trainium2 NeuronCores (Bass/Tile).

loss = clip(distmat * onehot(labels), 1e-12, 1e12).sum() / B
     = (sum_i clip(||x_i - c_{y_i}||^2) + B*(C-1)*1e-12) / B

Data-parallel over the batch: each of the 8 cores gets 4096 rows of x and
labels plus the full (replicated) centers table.  Per 128-row tile the core
DMAs x contiguously, gathers the 128 label-selected center rows with one
indirect DMA, computes (x-c) on the vector engine and Square+row-sum on the
scalar engine.  Per-sample distances are clipped on-device; the per-core
scalar partial sum is combined on the host (the sanctioned all-reduce).
"""

import numpy as np

BATCH, NUM_CLASSES, FEATURE_DIM = 32768, 1024, 256
N_CORES = 8
SHARD = BATCH // N_CORES  # 4096
P = 128
N_TILES = SHARD // P  # 32
CLAMP_MIN, CLAMP_MAX = 1e-12, 1e12

_CACHE: dict = {}


def _build_nc():
    import concourse.bass as bass
    import concourse.bacc as bacc
    import concourse.tile as tile
    from concourse import mybir

    f32 = mybir.dt.float32
    i32 = mybir.dt.int32

    nc = bacc.Bacc("TRN2", target_bir_lowering=False, debug=False)

    x_d = nc.dram_tensor("x", [SHARD, FEATURE_DIM], f32, kind="ExternalInput")
    # labels pre-transposed on host to [P, N_TILES]: lab[p, i] = labels[i*P + p]
    lab_d = nc.dram_tensor("labels", [P, N_TILES], i32, kind="ExternalInput")
    cen_d = nc.dram_tensor(
        "centers", [NUM_CLASSES, FEATURE_DIM], f32, kind="ExternalInput"
    )
    out_d = nc.dram_tensor("out", [1, 1], f32, kind="ExternalOutput")

    with tile.TileContext(nc) as tc:
        with (
            tc.tile_pool(name="sbuf", bufs=4) as sbuf,
            tc.tile_pool(name="single", bufs=1) as single,
            tc.tile_pool(name="psum", bufs=1, space="PSUM") as psum,
        ):
            lab_all = single.tile([P, N_TILES], i32)
            nc.sync.dma_start(out=lab_all[:], in_=lab_d[:, :])

            acc = single.tile([P, N_TILES], f32)
            for i in range(N_TILES):
                x_t = sbuf.tile([P, FEATURE_DIM], f32)
                nc.sync.dma_start(out=x_t[:], in_=x_d[i * P : (i + 1) * P, :])
                g_t = sbuf.tile([P, FEATURE_DIM], f32)
                nc.gpsimd.indirect_dma_start(
                    out=g_t[:],
                    out_offset=None,
                    in_=cen_d[:, :],
                    in_offset=bass.IndirectOffsetOnAxis(ap=lab_all[:, i : i + 1], axis=0),
                )
                d_t = sbuf.tile([P, FEATURE_DIM], f32)
                nc.vector.tensor_tensor(
                    out=d_t[:], in0=x_t[:], in1=g_t[:], op=mybir.AluOpType.subtract
                )
                s_t = sbuf.tile([P, FEATURE_DIM], f32)
                nc.scalar.activation(
                    out=s_t[:],
                    in_=d_t[:],
                    func=mybir.ActivationFunctionType.Square,
                    accum_out=acc[:, i : i + 1],
                )

            clipped = single.tile([P, N_TILES], f32)
            nc.vector.tensor_scalar(
                out=clipped[:],
                in0=acc[:],
                scalar1=float(CLAMP_MIN),
                scalar2=float(CLAMP_MAX),
                op0=mybir.AluOpType.max,
                op1=mybir.AluOpType.min,
            )
            rowsum = single.tile([P, 1], f32)
            nc.vector.reduce_sum(out=rowsum[:], in_=clipped[:], axis=mybir.AxisListType.X)

            ones = single.tile([P, 1], f32)
            nc.vector.memset(ones[:], 1.0)
            tot = psum.tile([1, 1], f32, space="PSUM")
            nc.tensor.matmul(out=tot[:], lhsT=rowsum[:], rhs=ones[:], start=True, stop=True)
            res = single.tile([1, 1], f32)
            nc.vector.tensor_copy(out=res[:], in_=tot[:])
            nc.sync.dma_start(out=out_d[:, :], in_=res[:])

    nc.finalize()
    return nc


def kernel(x: np.ndarray, centers: np.ndarray, labels: np.ndarray) -> np.ndarray:
    from concourse import bass_utils

    if "nc" not in _CACHE:
        _CACHE["nc"] = _build_nc()
    nc = _CACHE["nc"]

    x = np.ascontiguousarray(np.asarray(x, dtype=np.float32))
    centers = np.ascontiguousarray(np.asarray(centers, dtype=np.float32))
    lab = np.asarray(labels).astype(np.int32).reshape(N_CORES, N_TILES, P)

    xs = x.reshape(N_CORES, SHARD, FEATURE_DIM)
    in_maps = [
        {
            "x": np.ascontiguousarray(xs[c]),
            "labels": np.ascontiguousarray(lab[c].T),  # [P, N_TILES]
            "centers": centers,
        }
        for c in range(N_CORES)
    ]

    rr = bass_utils.run_bass_kernel_spmd(nc, in_maps, list(range(N_CORES)))
    _CACHE["last_results"] = rr

    total = sum(float(r["out"][0, 0]) for r in rr.results)
    loss = (total + BATCH * (NUM_CLASSES - 1) * CLAMP_MIN) / BATCH
    return np.asarray(loss, dtype=np.float32)
